# revision 1
# baseline (speedup 1.0000x reference)
"""Trainium2 Bass kernel for nn_GroupLinearEncoder.

Math (reference):
  h_b = feat_proj(x_b) = BN(einsum over l,c of x_b and w1_b, w2_b)   (N,1024)
  latent = 0.5*(bn(h0)+bn(h1))
  group_pred = (latent @ shared_w.T) @ embed_w.T + embed_b
  subj       = einsum(latent, fc_w[indices]) + b_sel
  out        = group_pred + subj @ embed_w.T + embed_b
             = (latent @ shared_w.T + subj) @ embed_w.T + 2*embed_b

Key algebraic folds used here:
  * group_pred + subj_res share the embed matmul: z = latent@shared_w.T + subj,
    out = z @ embed_w.T + 2*embed_b  -> embed_w is read ONCE.
  * Because every sample belongs to exactly one group, per-core
    cwt_i = shared_w.T + fc_w.T[:, group_i] applied to mask-selected samples
    and AllReduduced over cores yields z directly (shared term included).

Sharding over 8 cores:
  * feat_proj: data-parallel over batch (8 samples/core), AllGather h.
  * z: group-parallel (core i handles group i via sample masks), AllReduce.
  * embed: column-parallel over out_dim (4944 rows/core, padded), concat on host.
"""

import os
import sys

import numpy as np

N, H, P, KE = 64, 1024, 2048, 39548
PT = P // 128
NCORES = 8
NS = N // NCORES            # samples per core
L0, C0 = 257, 1024
L1, C1 = 197, 768
W = 4944                    # embed rows per core (8*4944 = 39552, 4 pad)
BN_EPS = 1e-5

_CACHE = {}


def _build_nc():
    if "/opt/trn_rl_repo" not in sys.path:
        sys.path.insert(0, "/opt/trn_rl_repo")
    import concourse.bass as bass
    import concourse.tile as tile
    from concourse import bacc, mybir
    from contextlib import ExitStack

    f32 = mybir.dt.float32
    bf16 = mybir.dt.bfloat16
    ALU = mybir.AluOpType
    ACTF = mybir.ActivationFunctionType

    nc = bacc.Bacc(num_devices=NCORES)

    KT = H // 128            # 8 k-tiles
    PT = P // 128            # 16 p-tiles
    NB0 = C0 // 128          # 8 c-chunks branch0
    NB1 = C1 // 128          # 6 c-chunks branch1

    x0t = nc.declare_dram_parameter("x0t", [C0, NS, L0], bf16, isOutput=False)
    x1t = nc.declare_dram_parameter("x1t", [C1, NS, L1], bf16, isOutput=False)
    w2_0t = nc.declare_dram_parameter("w2_0t", [C0, H], bf16, isOutput=False)
    w2_1t = nc.declare_dram_parameter("w2_1t", [C1, H], bf16, isOutput=False)
    w1_0 = nc.declare_dram_parameter("w1_0", [H, L0], f32, isOutput=False)
    w1_1 = nc.declare_dram_parameter("w1_1", [H, L1], f32, isOutput=False)
    gam0 = nc.declare_dram_parameter("gam0", [128, 8], f32, isOutput=False)
    bet0 = nc.declare_dram_parameter("bet0", [128, 8], f32, isOutput=False)
    gam1 = nc.declare_dram_parameter("gam1", [128, 8], f32, isOutput=False)
    bet1 = nc.declare_dram_parameter("bet1", [128, 8], f32, isOutput=False)
    cwt = nc.declare_dram_parameter("cwt", [H, P], bf16, isOutput=False)
    fcb = nc.declare_dram_parameter("fcb", [1, P], bf16, isOutput=False)
    maskrow = nc.declare_dram_parameter("maskrow", [1, N], bf16, isOutput=False)
    mask = nc.declare_dram_parameter("mask", [128, N], f32, isOutput=False)
    ewt = nc.declare_dram_parameter("ewt", [P, W], bf16, isOutput=False)
    eb2 = nc.declare_dram_parameter("eb2", [1, W], bf16, isOutput=False)
    out = nc.declare_dram_parameter("out", [N, W], f32, isOutput=True)

    with tile.TileContext(nc) as tc, ExitStack() as stack:
        singles = stack.enter_context(tc.tile_pool(name="singles", bufs=1))
        dpool = stack.enter_context(tc.tile_pool(name="dram", bufs=1, space="DRAM"))
        tpool = stack.enter_context(tc.tile_pool(name="touchp", bufs=2))
        _tn = [0]

        def touch(ap):
            # absorb a DMA's queue semaphores into DVE's vector clock so
            # downstream DVE ops need only engine-local ordering
            _tn[0] += 1
            tt = tpool.tile([ap.shape[0], 1], ap.dtype, tag="touch",
                            name=f"touch{_tn[0]}")
            nc.vector.tensor_copy(out=tt, in_=ap[:, 0:1])

        # --- resident small tensors ---
        h0sb = singles.tile([128, N], f32)       # col = kt*8 + n_local
        h1sb = singles.tile([128, N], f32)
        gam0sb = singles.tile([128, 8], f32)
        bet0sb = singles.tile([128, 8], f32)
        gam1sb = singles.tile([128, 8], f32)
        bet1sb = singles.tile([128, 8], f32)
        masksb = singles.tile([128, N], f32)
        epssb = singles.tile([128, 1], f32)
        nc.sync.dma_start(out=gam0sb, in_=gam0[:, :])
        nc.sync.dma_start(out=bet0sb, in_=bet0[:, :])
        nc.sync.dma_start(out=gam1sb, in_=gam1[:, :])
        nc.sync.dma_start(out=bet1sb, in_=bet1[:, :])
        nc.sync.dma_start(out=masksb, in_=mask[:, :])
        for _t in (gam0sb, bet0sb, gam1sb, bet1sb, masksb):
            touch(_t)
        nc.vector.memset(epssb, BN_EPS)

        # combined fc+shared weights, resident through stage C
        cwtp = stack.enter_context(tc.tile_pool(name="cwtp", bufs=1))
        cwsb = []
        for kt in range(KT):
            t = cwtp.tile([128, P], bf16, tag=f"cw{kt}", name=f"cw{kt}")
            nc.sync.dma_start(out=t, in_=cwt[kt * 128:(kt + 1) * 128, :])
            cwsb.append(t)

        ps_ctx = tc.tile_pool(name="ps", bufs=2, space="PSUM")
        pspool = ps_ctx.__enter__()

        # ---------------- stage A : feat_proj matmuls ----------------
        # branch 0: per-sample moving operand (N=257 >= 256 keeps f32r fast)
        with tc.tile_pool(name="br0", bufs=1) as br0:
            x0sb = []
            for ci in range(NB0):
                t = br0.tile([128, NS, L0], bf16, tag=f"x0_{ci}", name=f"x0_{ci}")
                nc.sync.dma_start(out=t, in_=x0t[ci * 128:(ci + 1) * 128, :, :])
                x0sb.append(t)
            w1sb = []
            for kt in range(KT):
                t = br0.tile([128, L0], f32, tag=f"w10_{kt}", name=f"w10_{kt}")
                nc.sync.dma_start(out=t, in_=w1_0[kt * 128:(kt + 1) * 128, :])
                touch(t)
                w1sb.append(t)

            for kt in range(KT):
                w2blk = []
                for ci in range(NB0):
                    t = br0.tile([128, 128], bf16, tag=f"w2b{ci}", bufs=2,
                                 name=f"w20b_{kt}_{ci}")
                    nc.sync.dma_start(
                        out=t, in_=w2_0t[ci * 128:(ci + 1) * 128,
                                         kt * 128:(kt + 1) * 128])
                    w2blk.append(t)
                for grp in range(2):
                    vs = []
                    for j in range(4):
                        v = pspool.tile([128, L0], f32, tag=f"v{j}", name=f"v0_{kt}_{grp}_{j}")
                        vs.append(v)
                    for ci in range(NB0):
                        lhs = w2blk[ci][:, :]
                        for j in range(4):
                            n = grp * 4 + j
                            nc.tensor.matmul(
                                out=vs[j][:, :],
                                lhsT=lhs,
                                rhs=x0sb[ci][:, n, :],
                                start=(ci == 0),
                                stop=(ci == NB0 - 1),
                            )
                    for j in range(4):
                        n = grp * 4 + j
                        col = kt * 8 + n
                        nc.vector.tensor_mul(vs[j][:, :], vs[j][:, :], w1sb[kt][:, :])
                        nc.vector.tensor_reduce(
                            out=h0sb[:, col:col + 1], in_=vs[j][:, :],
                            axis=mybir.AxisListType.X, op=ALU.add)

        # branch 1: two samples per moving operand (N=394 >= 256)
        with tc.tile_pool(name="br1", bufs=1) as br1:
            x1sb = []
            for ci in range(NB1):
                t = br1.tile([128, NS, L1], bf16, tag=f"x1_{ci}", name=f"x1_{ci}")
                nc.sync.dma_start(out=t, in_=x1t[ci * 128:(ci + 1) * 128, :, :])
                x1sb.append(t)
            w1sb1 = []
            for kt in range(KT):
                t = br1.tile([128, L1], f32, tag=f"w11_{kt}", name=f"w11_{kt}")
                nc.sync.dma_start(out=t, in_=w1_1[kt * 128:(kt + 1) * 128, :])
                touch(t)
                w1sb1.append(t)

            for kt in range(KT):
                w2blk1 = []
                for ci in range(NB1):
                    t = br1.tile([128, 128], bf16, tag=f"w2c{ci}", bufs=2,
                                 name=f"w21b_{kt}_{ci}")
                    nc.sync.dma_start(
                        out=t, in_=w2_1t[ci * 128:(ci + 1) * 128,
                                         kt * 128:(kt + 1) * 128])
                    w2blk1.append(t)
                for grp in range(2):
                    vps = []
                    for j in range(2):
                        v = pspool.tile([128, 2, L1], f32, tag=f"v{j}", name=f"v1_{kt}_{grp}_{j}")
                        vps.append(v)
                    for ci in range(NB1):
                        lhs = w2blk1[ci][:, :]
                        for j in range(2):
                            pj = grp * 2 + j
                            nc.tensor.matmul(
                                out=vps[j][:, :, :],
                                lhsT=lhs,
                                rhs=x1sb[ci][:, 2 * pj:2 * pj + 2, :],
                                start=(ci == 0),
                                stop=(ci == NB1 - 1),
                            )
                    for j in range(2):
                        pj = grp * 2 + j
                        for s in range(2):
                            n = 2 * pj + s
                            col = kt * 8 + n
                            nc.vector.tensor_mul(vps[j][:, s, :], vps[j][:, s, :],
                                                 w1sb1[kt][:, :])
                            nc.vector.tensor_reduce(
                                out=h1sb[:, col:col + 1], in_=vps[j][:, s, :],
                                axis=mybir.AxisListType.X, op=ALU.add)

        ps_ctx.__exit__(None, None, None)

        # ---------------- stage B : AllGather h + BatchNorm + latent ----------------
        hb_local = dpool.tile([2, 128, N], f32)
        nc.sync.dma_start(out=hb_local[0], in_=h0sb[:, :])
        nc.sync.dma_start(out=hb_local[1], in_=h1sb[:, :])
        hg = dpool.tile([NCORES, 2, 128, N], f32, addr_space="Shared")
        nc.gpsimd.collective_compute(
            "AllGather",
            ALU.bypass,
            replica_groups=[list(range(NCORES))],
            ins=[hb_local[:].opt()],
            outs=[hg[:].opt()],
        )

        # load gathered h: [128, core, branch, 64]
        hall = singles.tile([128, NCORES, 2, N], f32)
        for g in range(NCORES):
            for b in range(2):
                nc.sync.dma_start(out=hall[:, g, b, :], in_=hg[g, b, :, :])
                touch(hall[:, g, b, :])

        latsb = []      # per kt: [128, 64], col = sample g (global)
        stx = stack.enter_context(tc.tile_pool(name="stats", bufs=4))
        for kt in range(KT):
            lat = singles.tile([128, N], f32, tag=f"lat{kt}", name=f"lat{kt}")
            ab = []  # per-branch (a, negb) columns
            for b, (gsb, bsb) in enumerate(((gam0sb, bet0sb), (gam1sb, bet1sb))):
                hga = hall[:, :, b, kt * 8:(kt + 1) * 8]    # [128, g(8), n(8)]
                st = stx.tile([128, NCORES, 6], f32, tag="st", name=f"st{kt}_{b}")
                for g in range(NCORES):
                    nc.vector.bn_stats(out=st[:, g, :], in_=hga[:, g, :])
                mv = stx.tile([128, 2], f32, tag="mv", name=f"mv{kt}_{b}")
                nc.vector.bn_aggr(out=mv, in_=st)
                rst = stx.tile([128, 1], f32, tag="rst", name=f"rst{kt}_{b}")
                # rstd = 1/sqrt(var + eps)
                nc.scalar.activation(out=rst, in_=mv[:, 1:2], func=ACTF.Sqrt,
                                     bias=epssb, scale=1.0)
                nc.vector.reciprocal(out=rst, in_=rst)
                a = stx.tile([128, 1], f32, tag="a", name=f"a{kt}_{b}")
                nc.vector.tensor_mul(a, rst, gsb[:, kt:kt + 1])
                negb = stx.tile([128, 1], f32, tag="negb", name=f"negb{kt}_{b}")
                # negb = mu*a - beta_half
                nc.vector.tensor_mul(negb, mv[:, 0:1], a)
                nc.vector.tensor_sub(negb, negb, bsb[:, kt:kt + 1])
                ab.append((a, negb))
            nbsum = stx.tile([128, 1], f32, tag="nbsum", name=f"nbsum{kt}")
            nc.vector.tensor_add(nbsum, ab[0][1], ab[1][1])
            lat3 = lat[:].rearrange("p (g n) -> p g n", g=NCORES)
            scr = stx.tile([128, NCORES, 8], f32, tag="scr", name=f"scr{kt}")
            # lat = h0*a0 - nbsum
            nc.vector.tensor_scalar_mul(lat3, hall[:, :, 0, kt * 8:(kt + 1) * 8],
                                        ab[0][0][:, 0:1])
            nc.vector.tensor_scalar_sub(lat3, lat3, nbsum[:, 0:1])
            # lat += h1*a1
            nc.vector.tensor_scalar_mul(scr, hall[:, :, 1, kt * 8:(kt + 1) * 8],
                                        ab[1][0][:, 0:1])
            nc.vector.tensor_add(lat3, lat3, scr[:, :, :])
            latsb.append(lat)

        # ---------------- stage C : z partial = cwt.T @ (latent*mask) + fcb x mask ----------------
        lmsb = []
        for kt in range(KT):
            lm = singles.tile([128, N], bf16, tag=f"lm{kt}", name=f"lm{kt}")
            nc.vector.tensor_mul(lm, latsb[kt], masksb)
            lmsb.append(lm)

        zfc_local = dpool.tile([PT, 128, N], f32)
        with tc.tile_pool(name="zps", bufs=1, space="PSUM") as zps, \
             tc.tile_pool(name="csing", bufs=1) as csing:
            fcbsb = csing.tile([1, P], bf16)
            nc.sync.dma_start(out=fcbsb, in_=fcb[:, :])
            maskrsb = csing.tile([1, N], bf16)
            nc.sync.dma_start(out=maskrsb, in_=maskrow[:, :])
            zp = []
            for half in range(2):
                t = zps.tile([128, 8 * N], f32, tag=f"zp{half}", name=f"zp{half}")
                zp.append(t)
            for pt in range(PT):
                o = zp[pt // 8][:, (pt % 8) * N:(pt % 8 + 1) * N]
                for kt in range(KT):
                    nc.tensor.matmul(
                        out=o,
                        lhsT=cwsb[kt][:, pt * 128:(pt + 1) * 128],
                        rhs=lmsb[kt][:, :],
                        start=(kt == 0), stop=False,
                    )
                nc.tensor.matmul(
                    out=o, lhsT=fcbsb[:, pt * 128:(pt + 1) * 128],
                    rhs=maskrsb[:, :], start=False, stop=True)
            for half in range(2):
                zsbuf = csing.tile([128, 8 * N], f32, tag=f"zst{half}", name=f"zst{half}")
                nc.vector.tensor_copy(out=zsbuf, in_=zp[half][:, :])
                src = zsbuf[:].rearrange("p (t c) -> p t c", t=8)
                dst = zfc_local[half * 8:(half + 1) * 8].rearrange("t p c -> p t c")
                nc.sync.dma_start(out=dst, in_=src)

        zr = dpool.tile([PT, 128, N], f32, addr_space="Shared")
        nc.gpsimd.collective_compute(
            "AllReduce",
            ALU.add,
            replica_groups=[list(range(NCORES))],
            ins=[zfc_local[:].opt()],
            outs=[zr[:].opt()],
        )

        zsb = []
        for pt in range(PT):
            t = singles.tile([128, N], f32, tag=f"z{pt}", name=f"z{pt}")
            nc.sync.dma_start(out=t, in_=zr[pt, :, :])
            touch(t)
            tb = singles.tile([128, N], bf16, tag=f"zb{pt}", name=f"zb{pt}")
            nc.vector.tensor_copy(out=tb, in_=t)
            zsb.append(tb)

        # ---------------- stage D : out = z.T @ ewt + 2*eb ----------------
        NBLK = (W + 511) // 512
        with tc.tile_pool(name="ewp", bufs=2) as ewp, \
             tc.tile_pool(name="odp", bufs=2, space="PSUM") as odp, \
             tc.tile_pool(name="osp", bufs=3) as osp, \
             tc.tile_pool(name="dsing", bufs=1) as dsing:
            eb2sb = dsing.tile([1, W], bf16)
            nc.sync.dma_start(out=eb2sb, in_=eb2[:, :])
            ones1 = dsing.tile([1, N], bf16)
            nc.vector.memset(ones1, 1.0)
            for nb in range(NBLK):
                bs = nb * 512
                bw = min(512, W - bs)
                ewtiles = []
                for pc in range(PT):
                    t = ewp.tile([128, 512], bf16, tag=f"ew{pc}", name=f"ew{nb}_{pc}")
                    nc.sync.dma_start(out=t[:, :bw], in_=ewt[pc * 128:(pc + 1) * 128, bs:bs + bw])
                    ewtiles.append(t)
                od = odp.tile([N, 512], f32, tag="od", name=f"od{nb}")
                for pc in range(PT):
                    nc.tensor.matmul(
                        out=od[:, :bw],
                        lhsT=zsb[pc][:, :],
                        rhs=ewtiles[pc][:, :bw],
                        start=(pc == 0), stop=False,
                    )
                nc.tensor.matmul(
                    out=od[:, :bw],
                    lhsT=ones1[:, :],
                    rhs=eb2sb[:, bs:bs + bw],
                    start=False, stop=True,
                )
                osb = osp.tile([N, 512], f32, tag="osb", name=f"osb{nb}")
                nc.vector.tensor_copy(out=osb[:, :bw], in_=od[:, :bw])
                nc.sync.dma_start(out=out[:, bs:bs + bw], in_=osb[:, :bw])

    nc.compile()
    return nc


def _host_prep(x0, x1, w1_0, w2_0, gamma0, beta0, w1_1, w2_1, gamma1, beta1,
               shared_w, fc_w, fc_b, embed_w, embed_b, indices):
    import ml_dtypes
    f = np.float32
    bf = ml_dtypes.bfloat16
    x0t = np.ascontiguousarray(x0.transpose(2, 0, 1)).astype(bf)   # [1024, 64, 257]
    x1t = np.ascontiguousarray(x1.transpose(2, 0, 1)).astype(bf)   # [768, 64, 197]
    w2_0t = np.ascontiguousarray(w2_0.T).astype(bf)
    w2_1t = np.ascontiguousarray(w2_1.T).astype(bf)
    gam0 = np.ascontiguousarray((gamma0 * 0.5).reshape(8, 128).T, dtype=f)
    bet0 = np.ascontiguousarray((beta0 * 0.5).reshape(8, 128).T, dtype=f)
    gam1 = np.ascontiguousarray((gamma1 * 0.5).reshape(8, 128).T, dtype=f)
    bet1 = np.ascontiguousarray((beta1 * 0.5).reshape(8, 128).T, dtype=f)
    swt = shared_w.T.astype(f)                                    # [1024, 2048]
    fcwt = fc_w.T.astype(f)                                       # [1024, 16384]
    ewt_pad = np.zeros((P, NCORES * W), dtype=bf)
    ewt_pad[:, :KE] = embed_w.T.astype(bf)
    eb2_pad = np.zeros((1, NCORES * W), dtype=bf)
    eb2_pad[0, :KE] = (2.0 * embed_b).astype(bf)

    idx = np.asarray(indices).astype(np.int64)
    in_maps = []
    for i in range(NCORES):
        m = (idx == i).astype(f)
        in_maps.append({
            "x0t": np.ascontiguousarray(x0t[:, i * NS:(i + 1) * NS, :]),
            "x1t": np.ascontiguousarray(x1t[:, i * NS:(i + 1) * NS, :]),
            "w2_0t": w2_0t,
            "w2_1t": w2_1t,
            "w1_0": np.ascontiguousarray(w1_0, dtype=f),
            "w1_1": np.ascontiguousarray(w1_1, dtype=f),
            "gam0": gam0, "bet0": bet0, "gam1": gam1, "bet1": bet1,
            "cwt": np.ascontiguousarray(swt + fcwt[:, i * P:(i + 1) * P]).astype(bf),
            "fcb": np.ascontiguousarray(fc_b[i * P:(i + 1) * P].reshape(1, P)).astype(bf),
            "maskrow": np.ascontiguousarray(m.reshape(1, N)).astype(bf),
            "mask": np.ascontiguousarray(np.broadcast_to(m, (128, N))),
            "ewt": np.ascontiguousarray(ewt_pad[:, i * W:(i + 1) * W]),
            "eb2": np.ascontiguousarray(eb2_pad[:, i * W:(i + 1) * W]),
        })
    return in_maps


def kernel(**inputs):
    if "/opt/trn_rl_repo" not in sys.path:
        sys.path.insert(0, "/opt/trn_rl_repo")
    from concourse.bass_utils import run_bass_kernel_spmd

    in_maps = _host_prep(**inputs)
    if "nc" not in _CACHE:
        _CACHE["nc"] = _build_nc()
    nc = _CACHE["nc"]
    res = run_bass_kernel_spmd(nc, in_maps, core_ids=list(range(NCORES)))
    outs = [np.asarray(res.results[i]["out"]) for i in range(NCORES)]
    full = np.concatenate(outs, axis=1)[:, :KE]
    return np.ascontiguousarray(full, dtype=np.float32)


if __name__ == "__main__":
    sys.path.insert(0, os.path.dirname(os.path.abspath(__file__)))
    import reference
    inputs = {k: np.asarray(v) for k, v in reference.setup_inputs().items()}
    expected = np.asarray(reference.reference(**inputs))
    actual = kernel(**inputs)
    err = np.abs(actual - expected).max() / (np.abs(expected).max() + 1e-12)
    print("Relative error:", err)

